# revision 32
# baseline (speedup 1.0000x reference)
"""GQA attention block (RMSNorm + QKV proj + partial RoPE + causal GQA
attention + XSA correction + out proj) on 8 trn2 NeuronCores.

Sharding: 2 batches x 4 KV-groups (each core: 1 batch, 1 kv head, 4 q heads).
Each core computes a partial output (its 4 heads through its wo column slice);
the host sums the 4 bf16 partials per batch in f32.

Layout/schedule highlights:
- bf16 end-to-end (host casts inputs); all matmuls run at 1 cycle/row.
- RoPE dims are interleaved host-side (rows [0,32,1,33,...]) so rotate-half
  becomes an adjacent-partition swap done by one DVE stream_shuffle.
- Causal diagonal tiles are column-trimmed; the in-block triangle is masked
  by a constant [128,128] upper-tri mask multiply on DVE.
- Row statistics (rmsnorm sum, softmax denominator, XSA dot, v-norm) are
  computed as full-height PSUM tiles via ones[128,128] matmuls (same PE cost
  as a 1-row sum - cost scales with the moving dim only), so no
  cross-partition broadcast is ever needed.
- v is transposed to token-major by one hardware DMA-transpose per chunk.
- Schedule: chunk j+1's QKV/x^2 passes and chunk j-1's out-projection are
  filler work for chunk j's attention (which alone is Act-exp-latency-bound);
  each head's XSA tail is deferred past the next head's first steps.
- PSUM: 2 shared qkv/outproj banks, 3 score, 1 pv, 2 sum/dot; chunk 0
  spreads its 7 projection passes over all 8 banks to stream with the DMA.
"""

import sys

for _p in ("/opt/trn_rl_repo", "/root/.axon_site/_ro/trn_rl_repo"):
    if _p not in sys.path:
        sys.path.append(_p)

import numpy as np
import ml_dtypes

import concourse.bass as bass
import concourse.bacc as bacc
import concourse.mybir as mybir
import concourse.tile as tile
from concourse.bass_utils import run_bass_kernel_spmd

F32 = mybir.dt.float32
BF16 = mybir.dt.bfloat16

B, T, D = 2, 2048, 2048
NH, NKV, HD = 16, 4, 128
RD = 64                    # rope dims
NH_L = NH // NKV           # 4 q heads per core
EL = (NH_L + 2) * HD       # 768: q0..q3, k, v
TC = 512                   # token chunk
NTC = T // TC              # 4
DC = D // 128              # 16 contraction chunks
S128 = float(1.0 / np.sqrt(HD))
SWAP_MASK = [i ^ 1 for i in range(32)]   # swap adjacent partitions
ROPE_PERM = [r // 2 + (r % 2) * 32 for r in range(64)]  # interleave
EPS = 1e-6

_CACHE = {}


def _build_nc():
    nc = bacc.Bacc("TRN2", target_bir_lowering=False, debug=False)

    xT = nc.declare_dram_parameter("xT", [D, T], BF16, isOutput=False)
    wT = nc.declare_dram_parameter("wqkvT", [D, EL], BF16, isOutput=False)
    woT = nc.declare_dram_parameter("woT", [NH_L * HD, D], BF16, isOutput=False)
    csP = nc.declare_dram_parameter("cs", [128, T], BF16, isOutput=False)
    outp = nc.declare_dram_parameter("out", [T, D], BF16, isOutput=True)

    ACT = mybir.ActivationFunctionType

    with tile.TileContext(nc) as tc:
        with (
            nc.allow_low_precision(reason="bf16 pipeline; 2e-2 tolerance"),
            tc.tile_pool(name="singles", bufs=1) as sg,
            tc.tile_pool(name="stream", bufs=2) as st,
            tc.tile_pool(name="ps", bufs=1, space="PSUM") as ps,
        ):
            # ---- persistent tiles ----
            w_sb = sg.tile([128, DC * EL], BF16, tag="w")
            cos_sb = sg.tile([RD, T], BF16, tag="cos")
            sin_sb = sg.tile([RD, T], BF16, tag="sin")
            wo_all = sg.tile([128, NH_L * D], BF16, tag="wo")
            ones_sq = sg.tile([128, 128], BF16, tag="ones_sq")
            eps_t = sg.tile([128, 1], F32, tag="eps_t")
            mask_tri = sg.tile([128, 128], BF16, tag="mask_tri")
            khat = sg.tile([128, T], BF16, tag="khat")    # [hd, key tokens]
            vtok = sg.tile([128, T], BF16, tag="vtok")    # [key tokens, hd]

            nc.vector.memset(ones_sq, 1.0)
            nc.vector.memset(eps_t, EPS)
            nc.vector.memset(mask_tri, 1.0)
            # keep mask[r, c] = 1 iff c >= r (query col >= key row)
            nc.gpsimd.affine_select(
                out=mask_tri, in_=mask_tri,
                compare_op=mybir.AluOpType.is_ge,
                fill=0.0, base=0, pattern=[[1, 128]], channel_multiplier=-1,
            )

            # startup DMAs, one strided transfer each
            xt = [None] * NTC
            xt[0] = st.tile([128, DC * TC], BF16, tag="xt", bufs=2, name="xt0")
            for lo, hi in ((0, 1), (1, 4), (4, 8), (8, 12), (12, 16)):
                n = hi - lo
                nc.sync.dma_start(
                    out=xt[0][:, lo * TC:hi * TC].rearrange(
                        "p (i t) -> p i t", i=n),
                    in_=xT[lo * 128:hi * 128, 0:TC].rearrange(
                        "(i p) t -> p i t", p=128),
                )
                nc.sync.dma_start(
                    out=w_sb[:, lo * EL:hi * EL].rearrange(
                        "p (i e) -> p i e", i=n),
                    in_=wT[lo * 128:hi * 128, :].rearrange(
                        "(i p) e -> p i e", p=128),
                )
            nc.sync.dma_start(out=cos_sb, in_=csP[0:RD, :])
            nc.sync.dma_start(out=sin_sb, in_=csP[RD:128, :])
            nc.sync.dma_start(
                out=wo_all.rearrange("p (h d) -> p h d", h=NH_L),
                in_=woT[:, :].rearrange("(h p) d -> p h d", p=128),
            )

            qhat = [None] * NH_L   # current chunk's rope'd q heads
            vh_cur = [None] * NTC  # per-chunk v-hat [hd, tok]
            rvns = [None] * NTC
            aout = [[None] * NTC for _ in range(NH_L)]

            def emit_qkv_and_rope(j):
                js = slice(j * TC, (j + 1) * TC)
                # --- QKV + x^2: 7 accumulation passes through the shared
                # "acc" PSUM pair.  Pass order rs, k, v, q0..q3 so rope and
                # attention of this chunk unblock as early as possible.
                qr = [st.tile([128, TC], BF16, tag="qr", bufs=12,
                              name=f"qr{j}_{e}") for e in range(6)]
                # chunk 0 runs before any attention: spread its 7 passes
                # across the idle attention banks so they all stream
                # concurrently with the input DMA
                if j == 0:
                    ptag = {"rs": ("acc", 2), 4: ("sd", 2), 5: ("pv", 1),
                            0: ("sc", 3), 1: ("sc", 3), 2: ("acc", 2),
                            3: ("sd", 2)}
                else:
                    ptag = {k: ("acc", 2) for k in ("rs", 0, 1, 2, 3, 4, 5)}
                rsb = st.tile([128, TC], BF16, tag="rsb", bufs=2)
                # ones-matmul rs: its PE cost doubles as filler work
                # inside the Act-bound attention windows
                if True:
                    ps_sr = ps.tile([128, TC], F32, tag=ptag["rs"][0],
                                    bufs=ptag["rs"][1], name=f"pssr{j}")
                    for i in range(DC):
                        x2 = st.tile([128, TC], BF16, tag="x2", bufs=2,
                                     name=f"x2_{j}_{i}")
                        eng = nc.vector if i % 2 == 0 else nc.gpsimd
                        eng.tensor_mul(
                            x2, xt[j][:, i * TC:(i + 1) * TC],
                            xt[j][:, i * TC:(i + 1) * TC],
                        )
                        nc.tensor.matmul(ps_sr, ones_sq, x2,
                                         start=(i == 0), stop=(i == DC - 1))
                    sq_f = st.tile([128, TC], F32, tag="sq", bufs=2)
                    nc.scalar.activation(sq_f, ps_sr, ACT.Sqrt,
                                         scale=1.0 / D, bias=eps_t)
                    nc.vector.reciprocal(rsb, sq_f)
                for e in (4, 5, 0, 1, 2, 3):
                    ps_q = ps.tile([128, TC], F32, tag=ptag[e][0],
                                   bufs=ptag[e][1], name=f"psq{j}_{e}")
                    for i in range(DC):
                        nc.tensor.matmul(
                            ps_q,
                            w_sb[:, i * EL + e * 128: i * EL + (e + 1) * 128],
                            xt[j][:, i * TC:(i + 1) * TC],
                            start=(i == 0), stop=(i == DC - 1),
                        )
                    nc.scalar.copy(qr[e], ps_q)

                # --- rope + rs scaling for q heads and k ---
                cosj = cos_sb[:, js]
                sinj = sin_sb[:, js]   # interleaved, odd rows negated
                for t in (4, 0, 1, 2, 3):  # k first: unblocks next chunk
                    if t < NH_L:
                        dst = st.tile([128, TC], BF16, tag="qh", bufs=8,
                                      name=f"qh{j}_{t}")
                        qhat[t] = dst
                    else:
                        dst = khat[:, js]
                    nc.vector.tensor_mul(dst, qr[t], rsb)
                    shuf = st.tile([RD, TC], BF16, tag="t2s", bufs=2)
                    t1 = st.tile([RD, TC], BF16, tag="t1", bufs=2)
                    nc.vector.stream_shuffle(shuf, dst[0:RD], SWAP_MASK)
                    nc.vector.tensor_mul(shuf, shuf, sinj)
                    nc.vector.tensor_mul(t1, dst[0:RD], cosj)
                    nc.vector.tensor_add(dst[0:RD], t1, shuf)

                # --- v-hat, vns, token-major v ---
                vh = st.tile([128, TC], BF16, tag="vh", bufs=2,
                             name=f"vh{j}")
                vh_cur[j] = vh
                nc.vector.tensor_mul(vh, qr[5], rsb)
                vsq = st.tile([128, TC], BF16, tag="x2", bufs=2,
                              name=f"vsq{j}")
                nc.gpsimd.tensor_mul(vsq, vh, vh)
                ps_vns = ps.tile([128, TC], F32, tag="sc", bufs=3,
                                 name=f"psvns{j}")
                nc.tensor.matmul(ps_vns, ones_sq, vsq, start=True, stop=True)
                rv = st.tile([128, TC], BF16, tag="rvns", bufs=2,
                             name=f"rvns{j}")
                rvns[j] = rv
                nc.vector.reciprocal(rv, ps_vns)

                nc.sync.dma_start_transpose(
                    out=vtok[:, js].rearrange("p (b c) -> p b c", b=4),
                    in_=vh,
                )

            def emit_attention(j):
                js = slice(j * TC, (j + 1) * TC)
                nkt = 4 * (j + 1)

                def emit_xsa(h, ps_sd, ps_pv):
                    # normalization + XSA correction (full-height stats;
                    # aout = (pv - vh*(dot*rvns)) * inv).  The very last
                    # head gates the final out-projection, so its chain is
                    # emitted in column quarters to shorten the latency.
                    invb = st.tile([128, TC], BF16, tag="bc", bufs=4,
                                   name=f"invb{j}_{h}")
                    pvsb = st.tile([128, TC], BF16, tag="pvsb", bufs=2)
                    tu = st.tile([128, TC], BF16, tag="tu", bufs=2)
                    ps_dot = ps.tile([128, TC], F32, tag="sc", bufs=3,
                                     name=f"psdot{j}_{h}")
                    ub = st.tile([128, TC], BF16, tag="bc", bufs=4,
                                 name=f"ub{j}_{h}")
                    m2 = st.tile([128, TC], BF16, tag="m2", bufs=2)
                    dd = st.tile([128, TC], BF16, tag="dd", bufs=2)
                    ao = st.tile([128, TC], BF16, tag="ao", bufs=12,
                                 name=f"ao{j}_{h}")
                    aout[h][j] = ao
                    tail = (j == NTC - 1 and h == NH_L - 1)
                    qparts = ([slice(q * 128, (q + 1) * 128) for q in range(4)]
                              if tail else [slice(0, TC)])
                    for qs in qparts:
                        nc.vector.reciprocal(invb[:, qs], ps_sd[:, qs])
                        nc.scalar.copy(pvsb[:, qs], ps_pv[:, qs])
                        nc.vector.tensor_mul(tu[:, qs], pvsb[:, qs],
                                             vh_cur[j][:, qs])
                        nc.tensor.matmul(ps_dot[:, qs], ones_sq, tu[:, qs],
                                         start=True, stop=True)
                        nc.vector.tensor_mul(ub[:, qs], ps_dot[:, qs],
                                             rvns[j][:, qs])
                        nc.vector.tensor_mul(m2[:, qs], vh_cur[j][:, qs],
                                             ub[:, qs])
                        nc.vector.tensor_sub(dd[:, qs], pvsb[:, qs],
                                             m2[:, qs])
                        nc.vector.tensor_mul(ao[:, qs], dd[:, qs],
                                             invb[:, qs])

                pending = None
                for h in range(NH_L):
                    ps_sd = ps.tile([128, TC], F32, tag="sd", bufs=2,
                                    name=f"pssd{j}_{h}")
                    ps_pv = ps.tile([128, TC], F32, tag="pv", bufs=1,
                                    name=f"pspv{j}_{h}")
                    for kt in range(nkt):
                        jk, m = divmod(kt, 4)
                        diag = (jk == j)
                        qlo = m * 128 if diag else 0
                        ksl = slice(jk * TC + m * 128, jk * TC + (m + 1) * 128)
                        ps_sc = ps.tile([128, TC], F32, tag="sc", bufs=3,
                                        name=f"pssc{j}_{h}_{kt}")
                        nc.tensor.matmul(
                            ps_sc[:, qlo:], khat[:, ksl], qhat[h][:, qlo:],
                            start=True, stop=True,
                        )
                        pT = st.tile([128, TC], BF16, tag="pT", bufs=4)
                        nc.scalar.activation(pT[:, qlo:], ps_sc[:, qlo:],
                                             ACT.Exp, scale=S128)
                        if diag:
                            nc.vector.tensor_mul(
                                pT[:, qlo:qlo + 128],
                                pT[:, qlo:qlo + 128], mask_tri,
                            )
                        nc.tensor.matmul(
                            ps_sd[:, qlo:], ones_sq, pT[:, qlo:],
                            start=(kt == 0), stop=(kt == nkt - 1),
                        )
                        nc.tensor.matmul(
                            ps_pv[:, qlo:], vtok[:, ksl], pT[:, qlo:],
                            start=(kt == 0), stop=(kt == nkt - 1),
                        )
                        if kt == 1 and pending is not None:
                            # previous head's XSA tail, deferred so it does
                            # not head-of-line-block this head's first steps
                            emit_xsa(*pending)
                            pending = None
                    pending = (h, ps_sd, ps_pv)
                emit_xsa(*pending)

            def emit_outproj(j):
                js0 = j * TC
                last = (j == NTC - 1)
                # final chunk: attention banks are free, rotate through them
                tags = ("acc", "pv", "sd") if last else ("acc",)
                tagbufs = {"acc": 2, "pv": 1, "sd": 2}
                for tt in range(4):
                    stg = st.tile([128, D], BF16, tag="osb", bufs=3,
                                  name=f"ostg{j}_{tt}")
                    for m in range(4):
                        ms = slice(m * TC, (m + 1) * TC)
                        tg = tags[(tt * 4 + m) % len(tags)]
                        ps_o = ps.tile([128, TC], F32, tag=tg,
                                       bufs=tagbufs[tg],
                                       name=f"pso{j}_{m}_{tt}")
                        for h in range(NH_L):
                            nc.tensor.matmul(
                                ps_o,
                                aout[h][j][:, tt * 128:(tt + 1) * 128],
                                wo_all[:, h * D + m * TC: h * D + (m + 1) * TC],
                                start=(h == 0), stop=(h == NH_L - 1),
                            )
                        # gpsimd cannot read PSUM on HW: alternate Act/DVE
                        if (m + tt) % 2 == 0:
                            nc.scalar.copy(stg[:, ms], ps_o)
                        else:
                            nc.vector.tensor_copy(stg[:, ms], ps_o)
                        if last:
                            nc.sync.dma_start(
                                out=outp[js0 + tt * 128:
                                         js0 + (tt + 1) * 128, ms],
                                in_=stg[:, ms],
                            )
                    if not last:
                        nc.sync.dma_start(
                            out=outp[js0 + tt * 128: js0 + (tt + 1) * 128, :],
                            in_=stg,
                        )

            emit_qkv_and_rope(0)
            for j in range(NTC):
                # prefetch next chunk's activations during attention(j)
                if j + 1 < NTC:
                    xt[j + 1] = st.tile([128, DC * TC], BF16, tag="xt",
                                        bufs=2, name=f"xt{j + 1}")
                    for g in range(4):
                        gt = slice(g * 4 * TC, (g + 1) * 4 * TC)
                        nc.sync.dma_start(
                            out=xt[j + 1][:, gt].rearrange(
                                "p (i t) -> p i t", i=4),
                            in_=xT[g * 512:(g + 1) * 512,
                                   (j + 1) * TC:(j + 2) * TC].rearrange(
                                "(i p) t -> p i t", p=128),
                        )
                emit_attention(j)
                if j + 1 < NTC:
                    emit_qkv_and_rope(j + 1)
                # out-projection demoted one chunk: it becomes filler work
                # for the next chunk's attention (which alone is Act-bound)
                if j - 1 >= 0:
                    emit_outproj(j - 1)
            emit_outproj(NTC - 1)

    nc.compile()
    return nc


def _host_inputs(x, cos, sin, w_norm, wq, wk, wv, wo):
    """Build the 8 per-core input maps (host-side layout prep only)."""
    bf = ml_dtypes.bfloat16
    wn = w_norm.astype(np.float32)
    cosT = cos.T.astype(np.float32)                                # [64, T]
    sinT = sin.T.astype(np.float32)
    # interleaved rope layout: storage row 2r = dim r, row 2r+1 = dim r+32
    perm = np.array(ROPE_PERM)
    cos2 = cosT[perm]
    sin2 = sinT[perm]
    sin2[0::2] *= -1.0
    cs = np.ascontiguousarray(
        np.concatenate([cos2, sin2], axis=0)
    ).astype(bf)                                                   # [128, T]
    xTs = [np.ascontiguousarray(x[b].T).astype(bf) for b in range(B)]
    per_g = []
    for g in range(4):
        wq_s = (wq[g * NH_L * HD:(g + 1) * NH_L * HD] * wn[None, :]).copy()
        wk_s = (wk[g * HD:(g + 1) * HD] * wn[None, :]).copy()
        for hb in range(NH_L):
            blk = wq_s[hb * HD: hb * HD + RD]
            wq_s[hb * HD: hb * HD + RD] = blk[ROPE_PERM]
        wk_s[0:RD] = wk_s[0:RD][ROPE_PERM]
        wv_s = wv[g * HD:(g + 1) * HD] * wn[None, :]
        wqkvT = np.ascontiguousarray(
            np.concatenate([wq_s, wk_s, wv_s], axis=0).T
        ).astype(bf)                                               # [D, 768]
        woT_s = np.ascontiguousarray(
            wo[:, g * NH_L * HD:(g + 1) * NH_L * HD].T
        ).astype(bf)                                               # [512, D]
        per_g.append((wqkvT, woT_s))
    in_maps = []
    for c in range(8):
        b, g = divmod(c, 4)
        in_maps.append({
            "xT": xTs[b],
            "wqkvT": per_g[g][0],
            "woT": per_g[g][1],
            "cs": cs,
        })
    return in_maps


def kernel(x, cos, sin, w_norm, wq, wk, wv, wo, rope_dims=64, use_xsa=1,
           **_unused):
    if "nc" not in _CACHE:
        _CACHE["nc"] = _build_nc()
    nc = _CACHE["nc"]
    in_maps = _host_inputs(
        np.asarray(x), np.asarray(cos), np.asarray(sin), np.asarray(w_norm),
        np.asarray(wq), np.asarray(wk), np.asarray(wv), np.asarray(wo),
    )
    res_obj = run_bass_kernel_spmd(nc, in_maps, list(range(8)))
    _CACHE["last"] = res_obj
    res = res_obj.results
    out = np.zeros((B, T, D), dtype=np.float32)
    for c in range(8):
        b = c // 4
        out[b] += np.asarray(res[c]["out"], dtype=np.float32)
    return out


# revision 42
# speedup vs baseline: 1.0003x; 1.0003x over previous
"""GQA attention block (RMSNorm + QKV proj + partial RoPE + causal GQA
attention + XSA correction + out proj) on 8 trn2 NeuronCores.

Sharding: 2 batches x 4 KV-groups (each core: 1 batch, 1 kv head, 4 q heads).
Each core computes a partial output (its 4 heads through its wo column slice);
the host sums the 4 bf16 partials per batch in f32.

Layout/schedule highlights:
- bf16 end-to-end (host casts inputs); all matmuls run at 1 cycle/row.
- RoPE dims are interleaved host-side (rows [0,32,1,33,...]) so rotate-half
  becomes an adjacent-partition swap done by one DVE stream_shuffle.
- Causal diagonal tiles are column-trimmed; the in-block triangle is masked
  by a constant [128,128] upper-tri mask multiply on DVE.
- Row statistics (rmsnorm sum, softmax denominator, XSA dot, v-norm) are
  computed as full-height PSUM tiles via ones[128,128] matmuls (same PE cost
  as a 1-row sum - cost scales with the moving dim only), so no
  cross-partition broadcast is ever needed.
- v is transposed to token-major by one hardware DMA-transpose per chunk.
- Schedule: chunk j+1's QKV/x^2 passes and chunk j-1's out-projection are
  filler work for chunk j's attention (which alone is Act-exp-latency-bound);
  each head's XSA tail is deferred past the next head's first steps.
- PSUM: 2 shared qkv/outproj banks, 3 score, 1 pv, 2 sum/dot; chunk 0
  spreads its 7 projection passes over all 8 banks to stream with the DMA.
"""

import sys

for _p in ("/opt/trn_rl_repo", "/root/.axon_site/_ro/trn_rl_repo"):
    if _p not in sys.path:
        sys.path.append(_p)

import numpy as np
import ml_dtypes

import concourse.bass as bass
import concourse.bacc as bacc
import concourse.mybir as mybir
import concourse.tile as tile
from concourse.bass_utils import run_bass_kernel_spmd

F32 = mybir.dt.float32
BF16 = mybir.dt.bfloat16

B, T, D = 2, 2048, 2048
NH, NKV, HD = 16, 4, 128
RD = 64                    # rope dims
NH_L = NH // NKV           # 4 q heads per core
EL = (NH_L + 2) * HD       # 768: q0..q3, k, v
TC = 512                   # token chunk
NTC = T // TC              # 4
DC = D // 128              # 16 contraction chunks
S128 = float(1.0 / np.sqrt(HD))
SWAP_MASK = [i ^ 1 for i in range(32)]   # swap adjacent partitions
ROPE_PERM = [r // 2 + (r % 2) * 32 for r in range(64)]  # interleave
EPS = 1e-6

_CACHE = {}


def _build_nc():
    nc = bacc.Bacc("TRN2", target_bir_lowering=False, debug=False)

    xT = nc.declare_dram_parameter("xT", [D, T], BF16, isOutput=False)
    wT = nc.declare_dram_parameter("wqkvT", [D, EL], BF16, isOutput=False)
    woT = nc.declare_dram_parameter("woT", [NH_L * HD, D], BF16, isOutput=False)
    csP = nc.declare_dram_parameter("cs", [128, T], BF16, isOutput=False)
    outp = nc.declare_dram_parameter("out", [T, D], BF16, isOutput=True)

    ACT = mybir.ActivationFunctionType

    with tile.TileContext(nc) as tc:
        with (
            nc.allow_low_precision(reason="bf16 pipeline; 2e-2 tolerance"),
            tc.tile_pool(name="singles", bufs=1) as sg,
            tc.tile_pool(name="stream", bufs=2) as st,
            tc.tile_pool(name="ps", bufs=1, space="PSUM") as ps,
        ):
            # ---- persistent tiles ----
            w_sb = sg.tile([128, DC * EL], BF16, tag="w")
            cos_sb = sg.tile([RD, T], BF16, tag="cos")
            sin_sb = sg.tile([RD, T], BF16, tag="sin")
            wo_all = sg.tile([128, NH_L * D], BF16, tag="wo")
            ones_sq = sg.tile([128, 128], BF16, tag="ones_sq")
            eps_t = sg.tile([128, 1], F32, tag="eps_t")
            mask_tri = sg.tile([128, 128], BF16, tag="mask_tri")
            khat = sg.tile([128, T], BF16, tag="khat")    # [hd, key tokens]
            vtok = sg.tile([128, T], BF16, tag="vtok")    # [key tokens, hd]

            nc.vector.memset(ones_sq, 1.0)
            nc.vector.memset(eps_t, EPS)
            nc.vector.memset(mask_tri, 1.0)
            # keep mask[r, c] = 1 iff c >= r (query col >= key row)
            nc.gpsimd.affine_select(
                out=mask_tri, in_=mask_tri,
                compare_op=mybir.AluOpType.is_ge,
                fill=0.0, base=0, pattern=[[1, 128]], channel_multiplier=-1,
            )

            # startup DMAs, one strided transfer each
            xt = [None] * NTC
            xt[0] = st.tile([128, DC * TC], BF16, tag="xt", bufs=2, name="xt0")
            for lo, hi in ((0, 1), (1, 4), (4, 8), (8, 12), (12, 16)):
                n = hi - lo
                nc.sync.dma_start(
                    out=xt[0][:, lo * TC:hi * TC].rearrange(
                        "p (i t) -> p i t", i=n),
                    in_=xT[lo * 128:hi * 128, 0:TC].rearrange(
                        "(i p) t -> p i t", p=128),
                )
                nc.sync.dma_start(
                    out=w_sb[:, lo * EL:hi * EL].rearrange(
                        "p (i e) -> p i e", i=n),
                    in_=wT[lo * 128:hi * 128, :].rearrange(
                        "(i p) e -> p i e", p=128),
                )
            nc.sync.dma_start(out=cos_sb, in_=csP[0:RD, :])
            nc.sync.dma_start(out=sin_sb, in_=csP[RD:128, :])
            nc.sync.dma_start(
                out=wo_all.rearrange("p (h d) -> p h d", h=NH_L),
                in_=woT[:, :].rearrange("(h p) d -> p h d", p=128),
            )

            qhat = [None] * NH_L   # current chunk's rope'd q heads
            vh_cur = [None] * NTC  # per-chunk v-hat [hd, tok]
            rvns = [None] * NTC
            aout = [[None] * NTC for _ in range(NH_L)]

            def emit_qkv_and_rope(j):
                js = slice(j * TC, (j + 1) * TC)
                # --- QKV + x^2: 7 accumulation passes through the shared
                # "acc" PSUM pair.  Pass order rs, k, v, q0..q3 so rope and
                # attention of this chunk unblock as early as possible.
                qr = [st.tile([128, TC], BF16, tag="qr", bufs=12,
                              name=f"qr{j}_{e}") for e in range(6)]
                # chunk 0 runs before any attention: spread its 7 passes
                # across the idle attention banks so they all stream
                # concurrently with the input DMA
                if j == 0:
                    ptag = {"rs": ("acc", 2), 4: ("sd", 2), 5: ("pv", 1),
                            0: ("sc", 3), 1: ("sc", 3), 2: ("acc", 2),
                            3: ("sd", 2)}
                else:
                    ptag = {k: ("acc", 2) for k in ("rs", 0, 1, 2, 3, 4, 5)}
                rsb = st.tile([128, TC], BF16, tag="rsb", bufs=2)
                # ones-matmul rs: its PE cost doubles as filler work
                # inside the Act-bound attention windows
                if True:
                    ps_sr = ps.tile([128, TC], F32, tag=ptag["rs"][0],
                                    bufs=ptag["rs"][1], name=f"pssr{j}")
                    for i in range(DC):
                        x2 = st.tile([128, TC], BF16, tag="x2", bufs=2,
                                     name=f"x2_{j}_{i}")
                        eng = nc.vector if i % 2 == 0 else nc.gpsimd
                        eng.tensor_mul(
                            x2, xt[j][:, i * TC:(i + 1) * TC],
                            xt[j][:, i * TC:(i + 1) * TC],
                        )
                        nc.tensor.matmul(ps_sr, ones_sq, x2,
                                         start=(i == 0), stop=(i == DC - 1))
                    sq_f = st.tile([128, TC], F32, tag="sq", bufs=2)
                    nc.scalar.activation(sq_f, ps_sr, ACT.Sqrt,
                                         scale=1.0 / D, bias=eps_t)
                    nc.vector.reciprocal(rsb, sq_f)
                for e in (4, 5, 0, 1, 2, 3):
                    ps_q = ps.tile([128, TC], F32, tag=ptag[e][0],
                                   bufs=ptag[e][1], name=f"psq{j}_{e}")
                    for i in range(DC):
                        nc.tensor.matmul(
                            ps_q,
                            w_sb[:, i * EL + e * 128: i * EL + (e + 1) * 128],
                            xt[j][:, i * TC:(i + 1) * TC],
                            start=(i == 0), stop=(i == DC - 1),
                        )
                    nc.scalar.copy(qr[e], ps_q)

                # --- rope + rs scaling for q heads and k ---
                cosj = cos_sb[:, js]
                sinj = sin_sb[:, js]   # interleaved, odd rows negated
                for t in (4, 0, 1, 2, 3):  # k first: unblocks next chunk
                    if t < NH_L:
                        dst = st.tile([128, TC], BF16, tag="qh", bufs=8,
                                      name=f"qh{j}_{t}")
                        qhat[t] = dst
                    else:
                        dst = khat[:, js]
                    nc.vector.tensor_mul(dst, qr[t], rsb)
                    shuf = st.tile([RD, TC], BF16, tag="t2s", bufs=2)
                    t1 = st.tile([RD, TC], BF16, tag="t1", bufs=2)
                    nc.vector.stream_shuffle(shuf, dst[0:RD], SWAP_MASK)
                    nc.vector.tensor_mul(shuf, shuf, sinj)
                    nc.vector.tensor_mul(t1, dst[0:RD], cosj)
                    nc.vector.tensor_add(dst[0:RD], t1, shuf)

                # --- v-hat, vns, token-major v ---
                vh = st.tile([128, TC], BF16, tag="vh", bufs=2,
                             name=f"vh{j}")
                vh_cur[j] = vh
                nc.vector.tensor_mul(vh, qr[5], rsb)
                vsq = st.tile([128, TC], BF16, tag="x2", bufs=2,
                              name=f"vsq{j}")
                nc.gpsimd.tensor_mul(vsq, vh, vh)
                ps_vns = ps.tile([128, TC], F32, tag="sc", bufs=3,
                                 name=f"psvns{j}")
                nc.tensor.matmul(ps_vns, ones_sq, vsq, start=True, stop=True)
                rv = st.tile([128, TC], BF16, tag="rvns", bufs=2,
                             name=f"rvns{j}")
                rvns[j] = rv
                nc.vector.reciprocal(rv, ps_vns)

                nc.sync.dma_start_transpose(
                    out=vtok[:, js].rearrange("p (b c) -> p b c", b=4),
                    in_=vh,
                )

            def emit_attention(j):
                js = slice(j * TC, (j + 1) * TC)
                nkt = 4 * (j + 1)

                def emit_xsa(h, ps_sd, ps_pv):
                    # normalization + XSA correction (full-height stats;
                    # aout = (pv - vh*(dot*rvns)) * inv).  The very last
                    # head gates the final out-projection, so its chain is
                    # emitted in column quarters to shorten the latency.
                    invb = st.tile([128, TC], BF16, tag="bc", bufs=4,
                                   name=f"invb{j}_{h}")
                    pvsb = st.tile([128, TC], BF16, tag="pvsb", bufs=2)
                    tu = st.tile([128, TC], BF16, tag="tu", bufs=2)
                    ps_dot = ps.tile([128, TC], F32, tag="sc", bufs=3,
                                     name=f"psdot{j}_{h}")
                    ub = st.tile([128, TC], BF16, tag="bc", bufs=4,
                                 name=f"ub{j}_{h}")
                    m2 = st.tile([128, TC], BF16, tag="m2", bufs=2)
                    dd = st.tile([128, TC], BF16, tag="dd", bufs=2)
                    ao = st.tile([128, TC], BF16, tag="ao", bufs=12,
                                 name=f"ao{j}_{h}")
                    aout[h][j] = ao
                    tail = (j == NTC - 1 and h == NH_L - 1)
                    qparts = ([slice(q * 128, (q + 1) * 128) for q in range(4)]
                              if tail else [slice(0, TC)])
                    for qs in qparts:
                        nc.vector.reciprocal(invb[:, qs], ps_sd[:, qs])
                        nc.scalar.copy(pvsb[:, qs], ps_pv[:, qs])
                        nc.vector.tensor_mul(tu[:, qs], pvsb[:, qs],
                                             vh_cur[j][:, qs])
                        nc.tensor.matmul(ps_dot[:, qs], ones_sq, tu[:, qs],
                                         start=True, stop=True)
                        nc.vector.tensor_mul(ub[:, qs], ps_dot[:, qs],
                                             rvns[j][:, qs])
                        nc.vector.tensor_mul(m2[:, qs], vh_cur[j][:, qs],
                                             ub[:, qs])
                        nc.vector.tensor_sub(dd[:, qs], pvsb[:, qs],
                                             m2[:, qs])
                        nc.vector.tensor_mul(ao[:, qs], dd[:, qs],
                                             invb[:, qs])

                pending = None
                for h in range(NH_L):
                    ps_sd = ps.tile([128, TC], F32, tag="sd", bufs=2,
                                    name=f"pssd{j}_{h}")
                    ps_pv = ps.tile([128, TC], F32, tag="pv", bufs=1,
                                    name=f"pspv{j}_{h}")
                    for kt in range(nkt):
                        jk, m = divmod(kt, 4)
                        diag = (jk == j)
                        qlo = m * 128 if diag else 0
                        ksl = slice(jk * TC + m * 128, jk * TC + (m + 1) * 128)
                        ps_sc = ps.tile([128, TC], F32, tag="sc", bufs=3,
                                        name=f"pssc{j}_{h}_{kt}")
                        nc.tensor.matmul(
                            ps_sc[:, qlo:], khat[:, ksl], qhat[h][:, qlo:],
                            start=True, stop=True,
                        )
                        pT = st.tile([128, TC], BF16, tag="pT", bufs=6)
                        nc.scalar.activation(pT[:, qlo:], ps_sc[:, qlo:],
                                             ACT.Exp, scale=S128)
                        if diag:
                            nc.vector.tensor_mul(
                                pT[:, qlo:qlo + 128],
                                pT[:, qlo:qlo + 128], mask_tri,
                            )
                        nc.tensor.matmul(
                            ps_sd[:, qlo:], ones_sq, pT[:, qlo:],
                            start=(kt == 0), stop=(kt == nkt - 1),
                        )
                        nc.tensor.matmul(
                            ps_pv[:, qlo:], vtok[:, ksl], pT[:, qlo:],
                            start=(kt == 0), stop=(kt == nkt - 1),
                        )
                        if kt == 1 and pending is not None:
                            # previous head's XSA tail, deferred so it does
                            # not head-of-line-block this head's first steps
                            emit_xsa(*pending)
                            pending = None
                    pending = (h, ps_sd, ps_pv)
                emit_xsa(*pending)

            def emit_outproj(j):
                js0 = j * TC
                last = (j == NTC - 1)
                # final chunk: attention banks are free, rotate through them
                tags = ("acc", "pv", "sd") if last else ("acc",)
                tagbufs = {"acc": 2, "pv": 1, "sd": 2}
                for tt in range(4):
                    stg = st.tile([128, D], BF16, tag="osb", bufs=3,
                                  name=f"ostg{j}_{tt}")
                    for m in range(4):
                        ms = slice(m * TC, (m + 1) * TC)
                        tg = tags[(tt * 4 + m) % len(tags)]
                        ps_o = ps.tile([128, TC], F32, tag=tg,
                                       bufs=tagbufs[tg],
                                       name=f"pso{j}_{m}_{tt}")
                        for h in range(NH_L):
                            nc.tensor.matmul(
                                ps_o,
                                aout[h][j][:, tt * 128:(tt + 1) * 128],
                                wo_all[:, h * D + m * TC: h * D + (m + 1) * TC],
                                start=(h == 0), stop=(h == NH_L - 1),
                            )
                        # gpsimd cannot read PSUM on HW: alternate Act/DVE
                        if (m + tt) % 2 == 0:
                            nc.scalar.copy(stg[:, ms], ps_o)
                        else:
                            nc.vector.tensor_copy(stg[:, ms], ps_o)
                        if last:
                            nc.sync.dma_start(
                                out=outp[js0 + tt * 128:
                                         js0 + (tt + 1) * 128, ms],
                                in_=stg[:, ms],
                            )
                    if not last:
                        nc.sync.dma_start(
                            out=outp[js0 + tt * 128: js0 + (tt + 1) * 128, :],
                            in_=stg,
                        )

            emit_qkv_and_rope(0)
            for j in range(NTC):
                # prefetch next chunk's activations during attention(j)
                if j + 1 < NTC:
                    xt[j + 1] = st.tile([128, DC * TC], BF16, tag="xt",
                                        bufs=2, name=f"xt{j + 1}")
                    for g in range(4):
                        gt = slice(g * 4 * TC, (g + 1) * 4 * TC)
                        nc.sync.dma_start(
                            out=xt[j + 1][:, gt].rearrange(
                                "p (i t) -> p i t", i=4),
                            in_=xT[g * 512:(g + 1) * 512,
                                   (j + 1) * TC:(j + 2) * TC].rearrange(
                                "(i p) t -> p i t", p=128),
                        )
                emit_attention(j)
                if j + 1 < NTC:
                    emit_qkv_and_rope(j + 1)
                # out-projection demoted one chunk: it becomes filler work
                # for the next chunk's attention (which alone is Act-bound)
                if j - 1 >= 0:
                    emit_outproj(j - 1)
            emit_outproj(NTC - 1)

    nc.compile()
    return nc


def _host_inputs(x, cos, sin, w_norm, wq, wk, wv, wo):
    """Build the 8 per-core input maps (host-side layout prep only)."""
    bf = ml_dtypes.bfloat16
    wn = w_norm.astype(np.float32)
    cosT = cos.T.astype(np.float32)                                # [64, T]
    sinT = sin.T.astype(np.float32)
    # interleaved rope layout: storage row 2r = dim r, row 2r+1 = dim r+32
    perm = np.array(ROPE_PERM)
    cos2 = cosT[perm]
    sin2 = sinT[perm]
    sin2[0::2] *= -1.0
    cs = np.ascontiguousarray(
        np.concatenate([cos2, sin2], axis=0)
    ).astype(bf)                                                   # [128, T]
    xTs = [np.ascontiguousarray(x[b].T).astype(bf) for b in range(B)]
    per_g = []
    for g in range(4):
        wq_s = (wq[g * NH_L * HD:(g + 1) * NH_L * HD] * wn[None, :]).copy()
        wk_s = (wk[g * HD:(g + 1) * HD] * wn[None, :]).copy()
        for hb in range(NH_L):
            blk = wq_s[hb * HD: hb * HD + RD]
            wq_s[hb * HD: hb * HD + RD] = blk[ROPE_PERM]
        wk_s[0:RD] = wk_s[0:RD][ROPE_PERM]
        wv_s = wv[g * HD:(g + 1) * HD] * wn[None, :]
        wqkvT = np.ascontiguousarray(
            np.concatenate([wq_s, wk_s, wv_s], axis=0).T
        ).astype(bf)                                               # [D, 768]
        woT_s = np.ascontiguousarray(
            wo[:, g * NH_L * HD:(g + 1) * NH_L * HD].T
        ).astype(bf)                                               # [512, D]
        per_g.append((wqkvT, woT_s))
    in_maps = []
    for c in range(8):
        b, g = divmod(c, 4)
        in_maps.append({
            "xT": xTs[b],
            "wqkvT": per_g[g][0],
            "woT": per_g[g][1],
            "cs": cs,
        })
    return in_maps


def kernel(x, cos, sin, w_norm, wq, wk, wv, wo, rope_dims=64, use_xsa=1,
           **_unused):
    if "nc" not in _CACHE:
        _CACHE["nc"] = _build_nc()
    nc = _CACHE["nc"]
    in_maps = _host_inputs(
        np.asarray(x), np.asarray(cos), np.asarray(sin), np.asarray(w_norm),
        np.asarray(wq), np.asarray(wk), np.asarray(wv), np.asarray(wo),
    )
    res_obj = run_bass_kernel_spmd(nc, in_maps, list(range(8)))
    _CACHE["last"] = res_obj
    res = res_obj.results
    out = np.zeros((B, T, D), dtype=np.float32)
    for c in range(8):
        b = c // 4
        out[b] += np.asarray(res[c]["out"], dtype=np.float32)
    return out
